# revision 18
# baseline (speedup 1.0000x reference)
"""Trainium2 Bass kernel for a full attention block (B=4, S=2048, H=1024, 16 heads).

Sharding: 8 cores = (batch b = core//2) x (query-half qh = core%2).
Each core computes the complete pipeline for its 1024 query rows of batch b:
QKV projections, 16-head attention over all 2048 keys, output projection,
residual add and layernorm.  No cross-core communication; the host slices
inputs and concatenates the 8 disjoint output shards.

Device-side layout choices (established against the TRN2 cost model):
  - x is fed transposed (xT: [H, S]) so all three projections can contract
    over H on the partition dim.  Key/value order is rolled per-core so the
    core's own query rows are always columns 0..1023 (softmax is invariant
    to consistent K/V permutation).
  - Q and K are produced transposed ([head-dim, seq]) so scores come out as
    scoresT [keys, queries]; exp runs PSUM->SBUF on the scalar engine; the
    ctx matmul uses V as the stationary operand with an appended ones
    column, which yields the softmax denominator L for free as row 64.
  - Biases enter exactly via a ones row appended to xT (row 1024) and bias
    rows in the padded weights; bo enters via a ones row in the ctxT tile.
  - bf16 for x/W/Q/K/V/probs/ctx matmuls (fp32 PSUM accumulate), fp32 for
    softmax normalization, residual and layernorm math.
"""

import numpy as np
import ml_dtypes

B, S, H, NH, DH = 4, 2048, 1024, 16, 64
P = 128
NCORES = 8
SQ = 1024        # query rows per core
HP = 1152        # H padded with a bias ones-row block (9 * 128)
KO = 9           # contraction subtiles over HP
EPS = 1e-12

_CACHE = {}


def _build_program():
    from concourse import bacc, tile, mybir

    f32 = mybir.dt.float32
    bf16 = mybir.dt.bfloat16
    f32r = mybir.dt.float32r
    AF = mybir.ActivationFunctionType
    OP = mybir.AluOpType

    nc = bacc.Bacc("TRN2", target_bir_lowering=False, debug=False,
                   num_devices=NCORES)

    xT_d = nc.dram_tensor("xT", [HP, S], bf16, kind="ExternalInput").ap()
    xq_d = nc.dram_tensor("xq", [SQ, H], f32, kind="ExternalInput").ap()
    wqT_d = nc.dram_tensor("wqT", [HP, H], bf16, kind="ExternalInput").ap()
    wkT_d = nc.dram_tensor("wkT", [HP, H], bf16, kind="ExternalInput").ap()
    wvT_d = nc.dram_tensor("wvT", [HP, H], bf16, kind="ExternalInput").ap()
    woT_d = nc.dram_tensor("woT", [HP, H], bf16, kind="ExternalInput").ap()
    gam_d = nc.dram_tensor("gam", [P, H], f32, kind="ExternalInput").ap()
    bet_d = nc.dram_tensor("bet", [P, H], f32, kind="ExternalInput").ap()
    out_d = nc.dram_tensor("out", [SQ, H], f32, kind="ExternalOutput").ap()

    xT_r = xT_d.rearrange("(o p) s -> p o s", p=P)      # [128, 9, 2048]
    wqT_r = wqT_d.rearrange("(o p) j -> p o j", p=P)
    wkT_r = wkT_d.rearrange("(o p) j -> p o j", p=P)
    wvT_r = wvT_d.rearrange("(o p) j -> p o j", p=P)
    woT_r = woT_d.rearrange("(o p) j -> p o j", p=P)
    xq_r = xq_d.rearrange("(t p) j -> p t j", p=P)      # [128, 8, 1024]
    out_r = out_d.rearrange("(t p) j -> p t j", p=P)

    with tile.TileContext(nc) as tc:
        with tc.tile_pool(name="pers", bufs=1) as pers, \
             tc.tile_pool(name="probs", bufs=4) as probs_pool:
            KT = pers.tile([P, 8, S], bf16)       # [p, jt, s]; j = jt*128+p
            QT = pers.tile([P, 8, SQ], bf16)
            V = pers.tile([P, 16, NH, 66], bf16)  # [k%128, k//128, head, d|ones@64]
            CTX = pers.tile([P, KO, SQ], bf16)    # ctxT, subtile 8 = ones row
            GAM = pers.tile([P, H], f32)
            BET = pers.tile([P, H], f32)


            nc.sync.dma_start(GAM[:], gam_d[:])
            nc.sync.dma_start(BET[:], bet_d[:])
            nc.gpsimd.memset(V[:, :, :, 64:65], 1.0)
            nc.gpsimd.memset(CTX[:, 8, :], 0.0)
            nc.gpsimd.memset(CTX[0:1, 8, :], 1.0)

            # ---------------- projections ----------------
            with tc.tile_pool(name="proj", bufs=1) as projp, \
                 tc.tile_pool(name="wstr", bufs=3) as wstr, \
                 tc.tile_pool(name="wvstr", bufs=2) as wvstr, \
                 tc.tile_pool(name="ppsum", bufs=2, space="PSUM") as ppsum:
                XT = projp.tile([P, KO, S], bf16)
                nc.sync.dma_start(XT[:], xT_r[:])

                for jt in range(8):
                    wq_t = wstr.tile([P, KO, P], bf16, tag="w")
                    nc.sync.dma_start(wq_t[:], wqT_r[:, :, jt * P:(jt + 1) * P])
                    for sc in range(2):
                        ps = ppsum.tile([P, 512], f32, tag="pp")
                        for ko in range(KO):
                            nc.tensor.matmul(
                                ps[:], wq_t[:, ko, :],
                                XT[:, ko, sc * 512:(sc + 1) * 512],
                                start=(ko == 0), stop=(ko == KO - 1))
                        nc.vector.tensor_copy(
                            QT[:, jt, sc * 512:(sc + 1) * 512], ps[:])
                    wk_t = wstr.tile([P, KO, P], bf16, tag="w")
                    nc.sync.dma_start(wk_t[:], wkT_r[:, :, jt * P:(jt + 1) * P])
                    for sc in range(4):
                        ps = ppsum.tile([P, 512], f32, tag="pp")
                        for ko in range(KO):
                            nc.tensor.matmul(
                                ps[:], wk_t[:, ko, :],
                                XT[:, ko, sc * 512:(sc + 1) * 512],
                                start=(ko == 0), stop=(ko == KO - 1))
                        nc.vector.tensor_copy(
                            KT[:, jt, sc * 512:(sc + 1) * 512], ps[:])

                for jc in range(2):
                    wv_t = wvstr.tile([P, KO, 512], bf16, tag="wv")
                    nc.sync.dma_start(wv_t[:], wvT_r[:, :, jc * 512:(jc + 1) * 512])
                    for st in range(16):
                        ps = ppsum.tile([P, 512], f32, tag="pp")
                        for ko in range(KO):
                            nc.tensor.matmul(
                                ps[:], XT[:, ko, st * P:(st + 1) * P],
                                wv_t[:, ko, :],
                                start=(ko == 0), stop=(ko == KO - 1))
                        nc.vector.tensor_copy(
                            V[:, st, jc * 8:(jc + 1) * 8, 0:64],
                            ps[:].rearrange("p (h d) -> p h d", d=64))

            # ---------------- attention + epilogue ----------------
            with tc.tile_pool(name="spsum", bufs=2, space="PSUM") as spsum, \
                 tc.tile_pool(name="cpsum", bufs=2, space="PSUM") as cpsum, \
                 tc.tile_pool(name="epi", bufs=2) as epi, \
                 tc.tile_pool(name="wo", bufs=1) as wop:

                for pair in range(8):
                    hA, hB = 2 * pair, 2 * pair + 1
                    jt = pair
                    ctxps = {h: cpsum.tile([65, SQ], f32, tag="ctx",
                                           name=f"ctxps_{h}")
                             for h in (hA, hB)}
                    for kt in range(16):
                        pr = {h: probs_pool.tile([P, SQ], bf16, tag="pt",
                                                 name=f"pr_{h}_{kt}")
                              for h in (hA, hB)}
                        for c in range(2):
                            for h in (hA, hB):
                                base = (h % 2) * 64
                                sp = spsum.tile([P, 512], f32, tag="sp")
                                nc.tensor.matmul(
                                    sp[:],
                                    KT[base:base + 64, jt, kt * P:(kt + 1) * P],
                                    QT[base:base + 64, jt, c * 512:(c + 1) * 512],
                                    start=True, stop=True)
                                nc.scalar.activation(
                                    pr[h][:, c * 512:(c + 1) * 512], sp[:],
                                    AF.Exp)
                        for h in (hA, hB):
                            for c in range(2):
                                nc.tensor.matmul(
                                    ctxps[h][:, c * 512:(c + 1) * 512],
                                    V[:, kt, h, 0:65],
                                    pr[h][:, c * 512:(c + 1) * 512],
                                    start=(kt == 0), stop=(kt == 15))
                    # unnormalized ctxT -> SBUF; denominator row -> LALL.
                    # Odd heads live at partitions 64..127 of CTX; DVE lanes
                    # are partition-locked, so bounce through SBUF + DMA.
                    lpair = epi.tile([2, SQ], f32, tag="lpair",
                                     name=f"lpair_{pair}")
                    for h in (hA, hB):
                        base = (h % 2) * 64
                        lstage = epi.tile([65, SQ], f32, tag="lstage",
                                          name=f"lstage_{h}")
                        nc.vector.tensor_copy(lstage[64:65, :],
                                              ctxps[h][64:65, :])
                        nc.sync.dma_start(lpair[h - hA:h - hA + 1, :],
                                          lstage[64:65, :])
                        if base == 0:
                            nc.vector.tensor_copy(
                                CTX[0:64, jt, :], ctxps[h][0:64, :])
                        else:
                            cstage = epi.tile([64, SQ], bf16, tag="cstage",
                                              name=f"cstage_{h}")
                            nc.vector.tensor_copy(cstage[:], ctxps[h][0:64, :])
                            nc.sync.dma_start(CTX[64:128, jt, :], cstage[:])
                    lrpair = epi.tile([2, SQ], f32, tag="lrpair",
                                      name=f"lrpair_{pair}")
                    nc.vector.reciprocal(lrpair[:], lpair[:])
                    # replicate 1/L across the head's 64 partitions and
                    # normalize in place
                    for h in (hA, hB):
                        base = (h % 2) * 64
                        lr0 = epi.tile([1, SQ], f32, tag="lr0",
                                       name=f"lr0_{h}")
                        nc.sync.dma_start(lr0[:], lrpair[h - hA:h - hA + 1, :])
                        lrep = epi.tile([P, SQ], f32, tag="lrep",
                                        name=f"lrep_{h}")
                        nc.gpsimd.partition_broadcast(lrep[:], lr0[0:1, :])
                        nc.vector.tensor_tensor(
                            CTX[base:base + 64, jt, :],
                            CTX[base:base + 64, jt, :],
                            lrep[base:base + 64, :], OP.mult)

                # ---------------- output projection + layernorm ----------------
                WO = wop.tile([P, KO, H], bf16)
                nc.sync.dma_start(WO[:], woT_r[:])
                for qt in range(8):
                    xqt = epi.tile([P, H], f32, tag="xq")
                    nc.sync.dma_start(xqt[:], xq_r[:, qt, :])
                    tmp = epi.tile([P, H], f32, tag="tmp")
                    for jc in range(2):
                        hp = spsum.tile([P, 512], f32, tag="sp")
                        for ko in range(KO):
                            nc.tensor.matmul(
                                hp[:], CTX[:, ko, qt * P:(qt + 1) * P],
                                WO[:, ko, jc * 512:(jc + 1) * 512],
                                start=(ko == 0), stop=(ko == KO - 1))
                        nc.vector.tensor_tensor(
                            tmp[:, jc * 512:(jc + 1) * 512], hp[:],
                            xqt[:, jc * 512:(jc + 1) * 512], OP.add)
                    stats = epi.tile([P, 2, 6], f32, tag="st")
                    mv = epi.tile([P, 2], f32, tag="mv")
                    for c in range(2):
                        nc.vector.bn_stats(
                            stats[:, c, :], tmp[:, c * 512:(c + 1) * 512])
                    nc.vector.bn_aggr(mv[:], stats[:])
                    ve = epi.tile([P, 1], f32, tag="ve")
                    nc.vector.tensor_scalar_add(ve[:], mv[:, 1:2], float(EPS))
                    sd = epi.tile([P, 1], f32, tag="sd")
                    nc.scalar.activation(sd[:], ve[:], AF.Sqrt)
                    rstd = epi.tile([P, 1], f32, tag="rstd")
                    nc.vector.reciprocal(rstd[:], sd[:])
                    osb = epi.tile([P, H], f32, tag="osb")
                    nc.vector.tensor_scalar(
                        osb[:], tmp[:], mv[:, 0:1], rstd[:],
                        OP.subtract, OP.mult)
                    nc.vector.tensor_tensor(osb[:], osb[:], GAM[:], OP.mult)
                    nc.vector.tensor_tensor(osb[:], osb[:], BET[:], OP.add)
                    nc.sync.dma_start(out_r[:, qt, :], osb[:])

    nc.compile()
    return nc


def _get_program():
    if "nc" not in _CACHE:
        _CACHE["nc"] = _build_program()
    return _CACHE["nc"]


def _prep_inputs(input_tensor, Wq, bq, Wk, bk, Wv, bv, Wo, bo, gamma, beta):
    bf = ml_dtypes.bfloat16
    x = np.asarray(input_tensor, np.float32)

    def padw(w, b, scale=1.0):
        m = np.zeros((HP, H), np.float32)
        m[:H] = np.asarray(w, np.float32).T * scale
        m[H] = np.asarray(b, np.float32) * scale
        return m.astype(bf)

    wqT = padw(Wq, bq, 1.0 / np.sqrt(DH))
    wkT = padw(Wk, bk)
    wvT = padw(Wv, bv)
    woT = padw(Wo, bo)
    gam = np.ascontiguousarray(
        np.broadcast_to(np.asarray(gamma, np.float32), (P, H)))
    bet = np.ascontiguousarray(
        np.broadcast_to(np.asarray(beta, np.float32), (P, H)))

    in_maps = []
    for core in range(NCORES):
        b, qh = core // 2, core % 2
        xb = x[b]
        rolled = np.concatenate(
            [xb[qh * SQ:(qh + 1) * SQ], xb[(1 - qh) * SQ:(2 - qh) * SQ]], 0)
        xT = np.zeros((HP, S), np.float32)
        xT[:H] = rolled.T
        xT[H] = 1.0
        in_maps.append({
            "xT": xT.astype(bf),
            "xq": np.ascontiguousarray(xb[qh * SQ:(qh + 1) * SQ]),
            "wqT": wqT, "wkT": wkT, "wvT": wvT, "woT": woT,
            "gam": gam, "bet": bet,
        })
    return in_maps


def run(inputs, trace=False, tmpdir=None):
    from concourse.bass_utils import run_bass_kernel_spmd
    nc = _get_program()
    in_maps = _prep_inputs(**inputs)
    res = run_bass_kernel_spmd(nc, in_maps, list(range(NCORES)), trace=trace,
                               tmpdir=tmpdir)
    out = np.zeros((B, S, H), np.float32)
    for core in range(NCORES):
        b, qh = core // 2, core % 2
        out[b, qh * SQ:(qh + 1) * SQ] = res.results[core]["out"]
    return out, res


def kernel(**inputs):
    out, _ = run(inputs, trace=False)
    return out
